# revision 3
# baseline (speedup 1.0000x reference)
"""Multi-level block-diagonal sparse attention (AttMLR) on 8 TRN2 NeuronCores.

Sharding: head-parallel — core c owns heads (2c, 2c+1). Each core:
  1. computes qT/kT (scaled, [d, t] layout) and v ([t, d] layout) for its heads
     from a replicated x^T and its slice of Wqkv,
  2. computes causal multi-level scores S^T = kT.T-style tiles, exp, masks the
     diagonal tiles, and accumulates y^T = v.T @ p^T with a fused ones-column
     that yields the softmax denominator,
  3. AllToAll redistributes y^T pieces so core c holds all heads' dims for
     t-slice c, then computes out_slice = y_slice @ Wproj.
Host assembles the 8 [256, 1024] slices.

Level structure: RANKS [32, 16, 16] over head-dim prefixes [0:32), [32:48),
[48:64) with block sizes [2048, 1024, 512]. Blocks nest, so a (k_tile, q_block)
pair contracts over a prefix of the 64 dims: 64 if same 512-block, 48 if same
1024-block, else 32 (level-0 spans all of T). Per-level 1/(rank*3) scaling is
folded into Wq columns on the host.
"""

import numpy as np

import concourse.bass as bass
import concourse.mybir as mybir
from concourse import bacc
from concourse.bass_utils import run_bass_kernel_spmd
from concourse.tile import TileContext
from concourse.masks import make_identity

T = 2048
C = 1024
H = 16
D = 64
NCORES = 8
P = 128
NO = C // P          # 8 contraction chunks of 128
QB = 512             # q-block size (score-tile free dim)
NQB = T // QB        # 4 q-blocks
NKT = T // P         # 16 k-tiles
F32 = mybir.dt.float32
F32R = mybir.dt.float32r
EXP = mybir.ActivationFunctionType.Exp

_CACHE = {}


def _ki(i, j):
    """Contraction depth for score tile (k_tile i, q_block j)."""
    if i // 4 == j:
        return 64
    if i // 8 == j // 2:
        return 48
    return 32


def _build():
    nc = bacc.Bacc(None, target_bir_lowering=False, num_devices=NCORES)

    xT = nc.declare_dram_parameter("xT", [P, NO, T], F32R, isOutput=False)
    wq = nc.declare_dram_parameter("wq", [P, NO, P], F32R, isOutput=False)
    wk = nc.declare_dram_parameter("wk", [P, NO, P], F32R, isOutput=False)
    wv = nc.declare_dram_parameter("wv", [P, NO, P], F32R, isOutput=False)
    wproj = nc.declare_dram_parameter("wproj", [P, NO, C], F32R, isOutput=False)
    masks = nc.declare_dram_parameter("masks", [P, 4, QB], F32R, isOutput=False)
    vones = nc.declare_dram_parameter("vones", [P, 2, NKT], F32R, isOutput=False)
    out = nc.declare_dram_parameter("out", [P, 2, C], F32, isOutput=True)

    with TileContext(nc) as tc:
        with (
            tc.tile_pool(name="persist", bufs=1) as persist,
            tc.tile_pool(name="dram", bufs=1, space="DRAM") as dram,
        ):
            wq_sb = persist.tile([P, NO, P], F32R)
            wk_sb = persist.tile([P, NO, P], F32R)
            wv_sb = persist.tile([P, NO, P], F32R)
            wproj_sb = persist.tile([P, NO, C], F32R)
            masks_sb = persist.tile([P, 4, QB], F32R)
            qT_sb = persist.tile([P, T], F32R)
            kT_sb = persist.tile([P, T], F32R)
            # v in natural [t, d] layout; per (head, t_tile) a [128, 65] lhsT
            # whose last column is 1.0 (softmax denominator row).
            v_sb = persist.tile([P, 2, NKT, 65], F32R)
            yT_sb = persist.tile([P, T], F32)
            ident = persist.tile([P, P], F32)

            nc.sync.dma_start(wq_sb[:], wq[:])
            nc.sync.dma_start(wk_sb[:], wk[:])
            nc.sync.dma_start(wv_sb[:], wv[:])
            nc.sync.dma_start(wproj_sb[:], wproj[:])
            nc.sync.dma_start(masks_sb[:], masks[:])
            make_identity(nc, ident[:])
            nc.sync.dma_start(v_sb[:, :, :, 64], vones[:])

            # ---- Phase 1: qT/kT/vT projections + v transpose ----
            with (
                tc.tile_pool(name="ph1", bufs=1) as ph1,
                tc.tile_pool(name="ps1", bufs=4, space="PSUM") as ps1,
                tc.tile_pool(name="ps1t", bufs=2, space="PSUM") as ps1t,
            ):
                xT_sb = ph1.tile([P, NO, T], F32R)
                vT_sb = ph1.tile([P, T], F32)
                for o in range(NO):
                    nc.sync.dma_start(xT_sb[:, o, :], xT[:, o, :])

                for tb in range(NQB):
                    for w_sb, dst in ((wq_sb, qT_sb), (wk_sb, kT_sb), (wv_sb, vT_sb)):
                        ps = ps1.tile([P, QB], F32, tag="proj")
                        for o in range(NO):
                            nc.tensor.matmul(
                                ps[:],
                                w_sb[:, o, :],
                                xT_sb[:, o, bass.ts(tb, QB)],
                                start=(o == 0),
                                stop=(o == NO - 1),
                            )
                        nc.scalar.copy(dst[:, bass.ts(tb, QB)], ps[:])

                for tt in range(NKT):
                    pst = ps1t.tile([P, P], F32, tag="vtr")
                    nc.tensor.transpose(pst[:], vT_sb[:, bass.ts(tt, P)], ident[:])
                    for h in range(2):
                        nc.scalar.copy(
                            v_sb[:, h, tt, 0:64], pst[:, bass.ts(h, 64)]
                        )

            # ---- Phase 2: scores -> exp -> mask -> y^T accumulation ----
            with (
                tc.tile_pool(name="pt", bufs=3) as ptp,
                tc.tile_pool(name="nrm", bufs=2) as nrm,
                tc.tile_pool(name="ps2s", bufs=2, space="PSUM") as ps2s,
                tc.tile_pool(name="ps2y", bufs=2, space="PSUM") as ps2y,
            ):
                for j in range(NQB):
                    nkt = 4 * j + 4  # causal k-tiles for this q-block (even)
                    for h in range(2):
                        yps = ps2y.tile([65, QB], F32, tag="yps")
                        for pair in range(nkt // 2):
                            sps = ps2s.tile([P, 2 * QB], F32, tag="sps")
                            pt = ptp.tile([P, 2 * QB], F32R, tag="pt")
                            for half in range(2):
                                i = 2 * pair + half
                                ki = _ki(i, j)
                                nc.tensor.matmul(
                                    sps[:, bass.ts(half, QB)],
                                    kT_sb[h * D : h * D + ki, bass.ts(i, P)],
                                    qT_sb[h * D : h * D + ki, bass.ts(j, QB)],
                                    start=True,
                                    stop=True,
                                )
                            nc.scalar.activation(pt[:], sps[:], EXP)
                            for half in range(2):
                                i = 2 * pair + half
                                if i >= 4 * j:
                                    d = i - 4 * j
                                    nc.vector.tensor_mul(
                                        pt[:, bass.ts(half, QB)],
                                        pt[:, bass.ts(half, QB)],
                                        masks_sb[:, d, :],
                                    )
                            for half in range(2):
                                i = 2 * pair + half
                                nc.tensor.matmul(
                                    yps[:],
                                    v_sb[:, h, i, :],
                                    pt[:, bass.ts(half, QB)],
                                    start=(i == 0),
                                    stop=(i == nkt - 1),
                                )
                        rec = nrm.tile([1, QB], F32, tag="rec")
                        nc.vector.reciprocal(rec[:], yps[64:65, :])
                        bc = nrm.tile([64, QB], F32, tag="bc")
                        nc.gpsimd.partition_broadcast(bc[:], rec[:])
                        nc.vector.tensor_mul(
                            yT_sb[h * D : (h + 1) * D, bass.ts(j, QB)],
                            yps[0:64, :],
                            bc[:],
                        )

            # ---- Phase 3: AllToAll of y^T pieces ----
            a2a_in = dram.tile([NCORES, P, T // NCORES], F32)
            a2a_out = dram.tile([NCORES, P, T // NCORES], F32)
            for d in range(NCORES):
                nc.sync.dma_start(a2a_in[d], yT_sb[:, bass.ts(d, T // NCORES)])
            nc.gpsimd.collective_compute(
                "AllToAll",
                mybir.AluOpType.bypass,
                replica_groups=[list(range(NCORES))],
                ins=[a2a_in.opt()],
                outs=[a2a_out.opt()],
            )

            # ---- Phase 4: out_slice = y_slice @ Wproj ----
            with (
                tc.tile_pool(name="ph4", bufs=1) as ph4,
                tc.tile_pool(name="st4", bufs=2) as st4,
                tc.tile_pool(name="ps4", bufs=2, space="PSUM") as ps4,
            ):
                yTall = ph4.tile([P, NCORES, T // NCORES], F32R)
                for s in range(NCORES):
                    # f32 -> f32r cast DMA must go through gpsimd
                    nc.gpsimd.dma_start(yTall[:, s, :], a2a_out[s])
                for tt in range(2):
                    stage = st4.tile([P, C], F32, tag="stage")
                    for nb in range(2):
                        pso = ps4.tile([P, QB], F32, tag="pso")
                        for o in range(NO):
                            nc.tensor.matmul(
                                pso[:],
                                yTall[:, o, bass.ts(tt, P)],
                                wproj_sb[:, o, bass.ts(nb, QB)],
                                start=(o == 0),
                                stop=(o == NO - 1),
                            )
                        nc.scalar.copy(stage[:, bass.ts(nb, QB)], pso[:])
                    nc.sync.dma_start(out[:, tt, :], stage[:])

    nc.compile()
    return nc


def _prep_inputs(x, Wqkv, Wproj):
    x2 = np.ascontiguousarray(x.reshape(T, C))
    xT = np.ascontiguousarray(x2.T)                       # [C, T]
    xT_a = np.ascontiguousarray(xT.reshape(NO, P, T).transpose(1, 0, 2))

    # per-dim scale folded into Wq: 1/(rank*3) by level of (d % 64)
    colscale = np.empty(P, np.float32)
    for dd in range(P):
        r = dd % D
        colscale[dd] = 1.0 / (32 * 3) if r < 32 else 1.0 / (16 * 3)

    wproj_a = np.ascontiguousarray(Wproj.reshape(NO, P, C).transpose(1, 0, 2))

    kp = np.arange(P)[:, None]
    qf = np.arange(QB)[None, :]
    masks = np.stack(
        [(qf >= kp + P * d).astype(np.float32) for d in range(4)], axis=0
    )  # [4, 128, 512]
    masks_a = np.ascontiguousarray(masks.transpose(1, 0, 2))

    in_maps = []
    for c in range(NCORES):
        cs = slice(P * c, P * (c + 1))
        wq_c = Wqkv[:, cs] * colscale[None, :]
        wk_c = Wqkv[:, C:2 * C][:, cs]
        wv_c = Wqkv[:, 2 * C:][:, cs]
        in_maps.append(
            {
                "xT": xT_a,
                "wq": np.ascontiguousarray(
                    wq_c.reshape(NO, P, P).transpose(1, 0, 2)
                ).astype(np.float32),
                "wk": np.ascontiguousarray(
                    wk_c.reshape(NO, P, P).transpose(1, 0, 2)
                ).astype(np.float32),
                "wv": np.ascontiguousarray(
                    wv_c.reshape(NO, P, P).transpose(1, 0, 2)
                ).astype(np.float32),
                "wproj": wproj_a,
                "masks": masks_a,
                "vones": np.ones((P, 2, NKT), np.float32),
            }
        )
    return in_maps


def kernel(x, Wqkv, Wproj, _trace=False):
    x = np.asarray(x, np.float32)
    Wqkv = np.asarray(Wqkv, np.float32)
    Wproj = np.asarray(Wproj, np.float32)

    if "nc" not in _CACHE:
        _CACHE["nc"] = _build()
    nc = _CACHE["nc"]

    in_maps = _prep_inputs(x, Wqkv, Wproj)
    res = run_bass_kernel_spmd(
        nc, in_maps, list(range(NCORES)), trace=_trace
    )
    _CACHE["last_result"] = res

    full = np.empty((T, C), np.float32)
    for c in range(NCORES):
        oc = res.results[c]["out"]  # [128, 2, 1024]
        full[P * 2 * c : P * 2 * (c + 1)] = oc.transpose(1, 0, 2).reshape(2 * P, C)
    return full.reshape(1, T, C)


# revision 4
# speedup vs baseline: 1.3887x; 1.3887x over previous
"""Multi-level block-diagonal sparse attention (AttMLR) on 8 TRN2 NeuronCores.

Sharding: head-parallel — core c owns heads (2c, 2c+1). Each core:
  1. computes qT/kT (scaled, [d, t] layout) and v ([t, d] layout) for its heads
     from a replicated x^T and its slice of Wqkv,
  2. computes causal multi-level scores, exp (fused PSUM->SBUF), masks the
     diagonal tiles, and accumulates y^T = v.T @ p^T with a fused ones-column
     that yields the softmax denominator,
  3. AllToAll redistributes y^T pieces so core c holds all heads' dims for
     t-slice c, then computes out_slice = y_slice @ Wproj.
Host assembles the 8 [256, 1024] slices.

Matmul operands are bf16 (weights/activations quantized once on the host or at
the PSUM->SBUF copy); all accumulation, softmax scores and the normalization
stay fp32 in PSUM.

Level structure: RANKS [32, 16, 16] over head-dim prefixes [0:32), [32:48),
[48:64) with block sizes [2048, 1024, 512]. Blocks nest, so a (k_tile, q_block)
pair contracts over a prefix of the 64 dims: 64 if same 512-block, 48 if same
1024-block, else 32 (level-0 spans all of T). Per-level 1/(rank*3) scaling is
folded into Wq columns on the host (before bf16 quantization).
"""

import ml_dtypes
import numpy as np

import concourse.bass as bass
import concourse.mybir as mybir
from concourse import bacc
from concourse.bass_utils import run_bass_kernel_spmd
from concourse.tile import TileContext, add_dep_helper
from concourse.masks import make_identity

T = 2048
C = 1024
H = 16
D = 64
NCORES = 8
P = 128
NO = C // P          # 8 contraction chunks of 128
QB = 512             # q-block size (score-tile free dim)
NQB = T // QB        # 4 q-blocks
NKT = T // P         # 16 k-tiles
TS = T // NCORES     # 256, per-core output t-slice
F32 = mybir.dt.float32
BF16 = mybir.dt.bfloat16
NPBF16 = ml_dtypes.bfloat16
EXP = mybir.ActivationFunctionType.Exp

_CACHE = {}


def _ki(i, j):
    """Contraction depth for score tile (k_tile i, q_block j)."""
    if i // 4 == j:
        return 64
    if i // 8 == j // 2:
        return 48
    return 32


def _build():
    nc = bacc.Bacc(None, target_bir_lowering=False, num_devices=NCORES)

    xT = nc.declare_dram_parameter("xT", [P, NO, T], BF16, isOutput=False)
    wq = nc.declare_dram_parameter("wq", [P, NO, P], BF16, isOutput=False)
    wk = nc.declare_dram_parameter("wk", [P, NO, P], BF16, isOutput=False)
    wv = nc.declare_dram_parameter("wv", [P, NO, P], BF16, isOutput=False)
    wproj = nc.declare_dram_parameter("wproj", [P, NO, C], BF16, isOutput=False)
    masks = nc.declare_dram_parameter("masks", [P, 4, QB], BF16, isOutput=False)
    vones = nc.declare_dram_parameter("vones", [P, 2, NKT], BF16, isOutput=False)
    out = nc.declare_dram_parameter("out", [P, 2, C], F32, isOutput=True)

    with TileContext(nc) as tc:
        with (
            tc.tile_pool(name="persist", bufs=1) as persist,
            tc.tile_pool(name="pt", bufs=4) as ptp,
            tc.tile_pool(name="nrm", bufs=2) as nrm,
            tc.tile_pool(name="st4", bufs=2) as st4,
            tc.tile_pool(name="dram", bufs=1, space="DRAM") as dram,
        ):
            wq_sb = persist.tile([P, NO, P], BF16)
            wk_sb = persist.tile([P, NO, P], BF16)
            wv_sb = persist.tile([P, NO, P], BF16)
            wproj_sb = persist.tile([P, NO, C], BF16)
            masks_sb = persist.tile([P, 4, QB], BF16)
            qT_sb = persist.tile([P, T], BF16)
            kT_sb = persist.tile([P, T], BF16)
            # v in natural [t, d] layout; per (head, t_tile) a [128, 65] lhsT
            # whose last column is 1.0 (softmax denominator row).
            v_sb = persist.tile([P, 2, NKT, 65], BF16)
            yT_sb = persist.tile([P, T], BF16)
            ident = persist.tile([P, P], BF16)
            xT_sb = persist.tile([P, NO, T], BF16)
            vT_sb = persist.tile([P, T], BF16)
            yTall = persist.tile([P, NCORES, TS], BF16)

            nc.sync.dma_start(wq_sb[:], wq[:])
            nc.sync.dma_start(wk_sb[:], wk[:])
            nc.sync.dma_start(wv_sb[:], wv[:])
            nc.sync.dma_start(masks_sb[:], masks[:])
            nc.sync.dma_start(v_sb[:, :, :, 64], vones[:])
            make_identity(nc, ident[:])
            last_x = None
            for o in range(NO):
                last_x = nc.sync.dma_start(xT_sb[:, o, :], xT[:, o, :])
            # wproj is phase-4-only: keep it off the phase-1 DMA critical path
            wp_dma = nc.sync.dma_start(wproj_sb[:], wproj[:])
            add_dep_helper(last_x.ins, wp_dma.ins, sync=False,
                           reason="load wproj after xT")

            # ---- Phase 1: qT/kT/vT projections + v transpose ----
            with (
                tc.tile_pool(name="ps1", bufs=4, space="PSUM") as ps1,
                tc.tile_pool(name="ps1t", bufs=2, space="PSUM") as ps1t,
            ):
                for tb in range(NQB):
                    for w_sb, dst in ((wq_sb, qT_sb), (wk_sb, kT_sb), (wv_sb, vT_sb)):
                        ps = ps1.tile([P, QB], F32, tag="proj")
                        for o in range(NO):
                            nc.tensor.matmul(
                                ps[:],
                                w_sb[:, o, :],
                                xT_sb[:, o, bass.ts(tb, QB)],
                                start=(o == 0),
                                stop=(o == NO - 1),
                            )
                        nc.scalar.copy(dst[:, bass.ts(tb, QB)], ps[:])
                    for tt in range(4 * tb, 4 * tb + 4):
                        pst = ps1t.tile([P, P], BF16, tag="vtr")
                        nc.tensor.transpose(
                            pst[:], vT_sb[:, bass.ts(tt, P)], ident[:]
                        )
                        for h in range(2):
                            nc.scalar.copy(
                                v_sb[:, h, tt, 0:64], pst[:, bass.ts(h, 64)]
                            )

            # ---- Phase 2: scores -> exp -> mask -> y^T accumulation ----
            with (
                tc.tile_pool(name="ps2s", bufs=2, space="PSUM") as ps2s,
                tc.tile_pool(name="ps2y", bufs=2, space="PSUM") as ps2y,
            ):
                for j in range(NQB):
                    nkt = 4 * j + 4  # causal k-tiles for this q-block (even)
                    for h in range(2):
                        yps = ps2y.tile([65, QB], F32, tag="yps")
                        for pair in range(nkt // 2):
                            sps = ps2s.tile([P, 2 * QB], F32, tag="sps")
                            pt = ptp.tile([P, 2 * QB], BF16, tag="pt")
                            for half in range(2):
                                i = 2 * pair + half
                                ki = _ki(i, j)
                                nc.tensor.matmul(
                                    sps[:, bass.ts(half, QB)],
                                    kT_sb[h * D : h * D + ki, bass.ts(i, P)],
                                    qT_sb[h * D : h * D + ki, bass.ts(j, QB)],
                                    start=True,
                                    stop=True,
                                )
                            nc.scalar.activation(pt[:], sps[:], EXP)
                            for half in range(2):
                                i = 2 * pair + half
                                if i >= 4 * j:
                                    nc.vector.tensor_mul(
                                        pt[:, bass.ts(half, QB)],
                                        pt[:, bass.ts(half, QB)],
                                        masks_sb[:, i - 4 * j, :],
                                    )
                            for half in range(2):
                                i = 2 * pair + half
                                nc.tensor.matmul(
                                    yps[:],
                                    v_sb[:, h, i, :],
                                    pt[:, bass.ts(half, QB)],
                                    start=(i == 0),
                                    stop=(i == nkt - 1),
                                )
                        rec = nrm.tile([1, QB], F32, tag="rec")
                        nc.vector.reciprocal(rec[:], yps[64:65, :])
                        bc = nrm.tile([64, QB], F32, tag="bc")
                        nc.gpsimd.partition_broadcast(bc[:], rec[:])
                        with nc.allow_low_precision(reason="bf16 y for comms"):
                            nc.vector.tensor_mul(
                                yT_sb[h * D : (h + 1) * D, bass.ts(j, QB)],
                                yps[0:64, :],
                                bc[:],
                            )

            # ---- Phase 3: AllToAll of y^T pieces ----
            a2a_in = dram.tile([NCORES, P, TS], BF16)
            a2a_out = dram.tile([NCORES, P, TS], BF16)
            for d in range(NCORES):
                nc.sync.dma_start(a2a_in[d], yT_sb[:, bass.ts(d, TS)])
            nc.gpsimd.collective_compute(
                "AllToAll",
                mybir.AluOpType.bypass,
                replica_groups=[list(range(NCORES))],
                ins=[a2a_in.opt()],
                outs=[a2a_out.opt()],
            )
            for s in range(NCORES):
                nc.sync.dma_start(yTall[:, s, :], a2a_out[s])

            # ---- Phase 4: out_slice = y_slice @ Wproj ----
            with tc.tile_pool(name="ps4", bufs=2, space="PSUM") as ps4:
                for tt in range(2):
                    stage = st4.tile([P, C], F32, tag="stage")
                    for nb in range(2):
                        pso = ps4.tile([P, QB], F32, tag="pso")
                        for o in range(NO):
                            nc.tensor.matmul(
                                pso[:],
                                yTall[:, o, bass.ts(tt, P)],
                                wproj_sb[:, o, bass.ts(nb, QB)],
                                start=(o == 0),
                                stop=(o == NO - 1),
                            )
                        nc.scalar.copy(stage[:, bass.ts(nb, QB)], pso[:])
                    nc.sync.dma_start(out[:, tt, :], stage[:])

    nc.compile()
    return nc


def _prep_inputs(x, Wqkv, Wproj):
    x2 = np.ascontiguousarray(x.reshape(T, C))
    xT = np.ascontiguousarray(x2.T)                       # [C, T]
    xT_a = np.ascontiguousarray(
        xT.reshape(NO, P, T).transpose(1, 0, 2)
    ).astype(NPBF16)

    # per-dim scale folded into Wq: 1/(rank*3) by level of (d % 64)
    colscale = np.where(np.arange(P) % D < 32, 1.0 / 96, 1.0 / 48).astype(
        np.float32
    )

    wproj_a = np.ascontiguousarray(
        Wproj.reshape(NO, P, C).transpose(1, 0, 2)
    ).astype(NPBF16)

    kp = np.arange(P)[:, None]
    qf = np.arange(QB)[None, :]
    masks = np.stack(
        [(qf >= kp + P * d).astype(np.float32) for d in range(4)], axis=0
    )
    masks_a = np.ascontiguousarray(masks.transpose(1, 0, 2)).astype(NPBF16)
    vones_a = np.ones((P, 2, NKT), NPBF16)

    in_maps = []
    for c in range(NCORES):
        cs = slice(P * c, P * (c + 1))
        wq_c = Wqkv[:, cs] * colscale[None, :]
        wk_c = Wqkv[:, C : 2 * C][:, cs]
        wv_c = Wqkv[:, 2 * C :][:, cs]
        in_maps.append(
            {
                "xT": xT_a,
                "wq": np.ascontiguousarray(
                    wq_c.reshape(NO, P, P).transpose(1, 0, 2)
                ).astype(NPBF16),
                "wk": np.ascontiguousarray(
                    wk_c.reshape(NO, P, P).transpose(1, 0, 2)
                ).astype(NPBF16),
                "wv": np.ascontiguousarray(
                    wv_c.reshape(NO, P, P).transpose(1, 0, 2)
                ).astype(NPBF16),
                "wproj": wproj_a,
                "masks": masks_a,
                "vones": vones_a,
            }
        )
    return in_maps


def kernel(x, Wqkv, Wproj, _trace=False):
    x = np.asarray(x, np.float32)
    Wqkv = np.asarray(Wqkv, np.float32)
    Wproj = np.asarray(Wproj, np.float32)

    if "nc" not in _CACHE:
        _CACHE["nc"] = _build()
    nc = _CACHE["nc"]

    in_maps = _prep_inputs(x, Wqkv, Wproj)
    res = run_bass_kernel_spmd(nc, in_maps, list(range(NCORES)), trace=_trace)
    _CACHE["last_result"] = res

    full = np.empty((T, C), np.float32)
    for c in range(NCORES):
        oc = res.results[c]["out"]  # [128, 2, 1024]
        full[2 * P * c : 2 * P * (c + 1)] = oc.transpose(1, 0, 2).reshape(
            2 * P, C
        )
    return full.reshape(1, T, C)
